# revision 15
# baseline (speedup 1.0000x reference)
"""BAD-descriptor kernel for Trainium2 (8 NeuronCores, SPMD over pairs).

Math: the reference gathers from an integral image at
  cy = clip(h + off_y, 0, H-1).astype(int) + r,  y0/y1 = cy -/+ rad(+1)
Because h is an integer grid, clip(h+off).astype(int) == clip(h + floor(off), 0, H-1),
so each box-mean term is just the radius-d box-mean image sampled at a clamped
integer 2D shift.  With only 3 radii we precompute, per batch b and d in {1,2,3},
the box-mean image BM_d (edge-replicate semantics of the reference integral image),
pad it by 16 with edge replication into BMP_d [256,256], and then

  out[b,p] = BMP_{d_p}[b][sy1:sy1+224, sx1:sx1+224]
           - BMP_{d_p}[b][sy2:sy2+224, sx2:sx2+224] - thr_p,
  sy = floor(off_y)+16 in [0,32], sx likewise.

Per-core device program (32 pairs/core):
  A) pair prep: floor/clip arithmetic on the offset vectors (DVE), producing
     int32 row/col window offsets in SBUF + negated thresholds broadcast
     across partitions.
  B) box-mean precompute: horizontal (2d+1)-taps via DVE shifted adds on
     column-padded x, vertical taps via PE matmul with constant band matrices
     (passed as input constants), scaled 1/area on ACT, column/row replicate
     padding, DMA into a DRAM scratch bmp[2,768,256].
  C) main loop over (p, b): two dynamic-offset HWDGE window DMAs (registers
     loaded from SBUF with values_load), one fused DVE op
     (W1 + (-thr)) - W2, one DMA to the output.
"""

import sys

sys.path.insert(0, "/opt/trn_rl_repo")

import numpy as np

import concourse.bass as bass
import concourse.bacc as bacc
import concourse.mybir as mybir
import concourse.tile as tile
from concourse.bass import ds
from concourse.bass_utils import run_bass_kernel_spmd

B = 2
H = W = 224
P_TOTAL = 256
N_CORES = 8
P_CORE = P_TOTAL // N_CORES  # 32
PAD = 16
RMAX = 3
HP = H + 2 * PAD  # 256 padded image rows
F32 = mybir.dt.float32
I32 = mybir.dt.int32

# window tile: 2 image rows per partition -> [112, 448] ([112, 2, 224] view)
NPART = 112
NFREE = (H * W) // NPART  # 448


def _band_matrices() -> np.ndarray:
    """sdt[d-1][r, h] = #{i in [-d,d] : clip(h+i, 0, H-1) == r}  (S_d transposed)."""
    sdt = np.zeros((3, H, H), np.float32)
    for d in (1, 2, 3):
        for h in range(H):
            for i in range(-d, d + 1):
                r = min(max(h + i, 0), H - 1)
                sdt[d - 1][r, h] += 1.0
    return sdt


def build_device_program(nc: bacc.Bacc):
    x_ap = nc.dram_tensor("x", [B, H, W], F32, kind="ExternalInput").ap()
    offy1_ap = nc.dram_tensor("offy1", [1, P_CORE], F32, kind="ExternalInput").ap()
    offx1_ap = nc.dram_tensor("offx1", [1, P_CORE], F32, kind="ExternalInput").ap()
    offy2_ap = nc.dram_tensor("offy2", [1, P_CORE], F32, kind="ExternalInput").ap()
    offx2_ap = nc.dram_tensor("offx2", [1, P_CORE], F32, kind="ExternalInput").ap()
    radii_ap = nc.dram_tensor("radii", [1, P_CORE], I32, kind="ExternalInput").ap()
    thr_ap = nc.dram_tensor("thr", [1, P_CORE], F32, kind="ExternalInput").ap()
    sdt_ap = nc.dram_tensor("sdt", [3, H, H], F32, kind="ExternalInput").ap()
    out_ap = nc.dram_tensor("out", [B, P_CORE, H, W], F32, kind="ExternalOutput").ap()

    with tile.TileContext(nc) as tc:
        build_kernel(tc, out_ap, x_ap, offy1_ap, offx1_ap, offy2_ap, offx2_ap,
                     radii_ap, thr_ap, sdt_ap)
    return nc


def build_kernel(tc, out_ap, x_ap, offy1_ap, offx1_ap, offy2_ap, offx2_ap,
                 radii_ap, thr_ap, sdt_ap):
    nc = tc.nc
    EngT = mybir.EngineType
    Alu = mybir.AluOpType
    Act = mybir.ActivationFunctionType

    from contextlib import ExitStack
    ctx = ExitStack()
    const_pool = ctx.enter_context(tc.tile_pool(name="const", bufs=1))
    work_pool = ctx.enter_context(tc.tile_pool(name="work", bufs=1))
    psum_pool = ctx.enter_context(tc.tile_pool(name="psum", bufs=2, space="PSUM"))
    dram_pool = ctx.enter_context(tc.tile_pool(name="dram", bufs=1, space="DRAM"))
    slab_pool = ctx.enter_context(tc.tile_pool(name="slab", bufs=10))
    o_pool = ctx.enter_context(tc.tile_pool(name="outt", bufs=6))

    # ---------------- Stage A: pair prep ----------------
    # load the small vectors into one-partition tiles
    vecs = {}
    for name, ap in (("offy1", offy1_ap), ("offx1", offx1_ap),
                     ("offy2", offy2_ap), ("offx2", offx2_ap), ("thr", thr_ap)):
        t = const_pool.tile([1, P_CORE], F32, tag=f"v_{name}")
        nc.sync.dma_start(out=t[:], in_=ap[:])
        vecs[name] = t
    radii_t = const_pool.tile([1, P_CORE], I32, tag="v_radii")
    nc.sync.dma_start(out=radii_t[:], in_=radii_ap[:])

    radf = const_pool.tile([1, P_CORE], F32, tag="radf")
    nc.vector.tensor_copy(out=radf[:], in_=radii_t[:])
    # clamp radius to [1,3] for safety
    nc.vector.tensor_scalar(out=radf[:], in0=radf[:], scalar1=1.0, scalar2=3.0,
                            op0=Alu.max, op1=Alu.min)

    def floor_to_base(off_t, name):
        """return [1,P_CORE] f32 tile with clip(floor(off),-16,16)+16 in [0,32]."""
        ti = const_pool.tile([1, P_CORE], I32, tag=f"fi_{name}")
        tf = const_pool.tile([1, P_CORE], F32, tag=f"ff_{name}")
        gt = const_pool.tile([1, P_CORE], F32, tag=f"gt_{name}")
        res = const_pool.tile([1, P_CORE], F32, tag=f"fl_{name}")
        nc.vector.tensor_copy(out=ti[:], in_=off_t[:])   # cast (round or trunc)
        nc.vector.tensor_copy(out=tf[:], in_=ti[:])      # back to f32, exact
        nc.vector.tensor_tensor(out=gt[:], in0=tf[:], in1=off_t[:], op=Alu.is_gt)
        nc.vector.tensor_tensor(out=res[:], in0=tf[:], in1=gt[:], op=Alu.subtract)
        # + PAD then clamp to [0, 2*PAD]
        nc.vector.tensor_scalar_add(out=res[:], in0=res[:], scalar1=float(PAD))
        nc.vector.tensor_scalar(out=res[:], in0=res[:], scalar1=0.0,
                                scalar2=float(2 * PAD), op0=Alu.max, op1=Alu.min)
        return res

    sy1 = floor_to_base(vecs["offy1"], "y1")
    sx1 = floor_to_base(vecs["offx1"], "x1")
    sy2 = floor_to_base(vecs["offy2"], "y2")
    sx2 = floor_to_base(vecs["offx2"], "x2")

    # row index into bmp[b]: (d-1)*256 + sy   (f32 arithmetic, exact)
    dbase = const_pool.tile([1, P_CORE], F32, tag="dbase")
    nc.vector.tensor_scalar(out=dbase[:], in0=radf[:], scalar1=1.0, scalar2=float(HP),
                            op0=Alu.subtract, op1=Alu.mult)
    row1f = const_pool.tile([1, P_CORE], F32, tag="row1f")
    row2f = const_pool.tile([1, P_CORE], F32, tag="row2f")
    nc.vector.tensor_tensor(out=row1f[:], in0=dbase[:], in1=sy1[:], op=Alu.add)
    nc.vector.tensor_tensor(out=row2f[:], in0=dbase[:], in1=sy2[:], op=Alu.add)

    row1 = const_pool.tile([1, P_CORE], I32, tag="row1")
    row2 = const_pool.tile([1, P_CORE], I32, tag="row2")
    col1 = const_pool.tile([1, P_CORE], I32, tag="col1")
    col2 = const_pool.tile([1, P_CORE], I32, tag="col2")
    nc.vector.tensor_copy(out=row1[:], in_=row1f[:])
    nc.vector.tensor_copy(out=row2[:], in_=row2f[:])
    nc.vector.tensor_copy(out=col1[:], in_=sx1[:])
    nc.vector.tensor_copy(out=col2[:], in_=sx2[:])

    # negated thresholds broadcast to all partitions: thr_bc[k, p] = -thr[p]
    thrneg = const_pool.tile([1, P_CORE], F32, tag="thrneg")
    nc.vector.tensor_scalar_mul(out=thrneg[:], in0=vecs["thr"][:], scalar1=-1.0)
    thr_bc = const_pool.tile([NPART, P_CORE], F32, tag="thr_bc")
    nc.gpsimd.partition_broadcast(thr_bc[:], thrneg[0:1, :], NPART)

    # ---------------- Stage B: box-mean precompute ----------------
    # sdt constants in SBUF. PE matmul operands must start at partition base
    # 0/32/64, so the second row-tile covers x rows 96..223 (32-row overlap
    # with the first) and all K-chunks stay 32-aligned.
    sdt_lo = const_pool.tile([128, 3, H], F32, tag="sdt_lo")
    sdt_hi = const_pool.tile([128, 3, H], F32, tag="sdt_hi")
    nc.sync.dma_start(out=sdt_lo[:], in_=sdt_ap[:, 0:128, :].rearrange("d r h -> r d h"))
    nc.sync.dma_start(out=sdt_hi[:], in_=sdt_ap[:, 96:224, :].rearrange("d r h -> r d h"))

    # bmp scratch in DRAM: [B, 3*HP, HP]
    bmp = dram_pool.tile([B, 3 * HP, HP], F32, tag="bmp")

    part_rows = ((0, 128), (96, 128))  # (row0, nrows) x-row tiles (overlapping)

    for b in range(B):
        # column-padded x tiles: xt[j] [nrows, 230]
        xts = []
        for j, (r0, nr) in enumerate(part_rows):
            xt = work_pool.tile([nr, W + 2 * RMAX], F32, tag=f"xt_{b}_{j}")
            nc.sync.dma_start(out=xt[:, RMAX:RMAX + W], in_=x_ap[b, r0:r0 + nr, :])
            nc.sync.dma_start(out=xt[:, RMAX - 1:RMAX], in_=x_ap[b, r0:r0 + nr, 0:1])
            nc.sync.dma_start(out=xt[:, RMAX + W:RMAX + W + 1],
                              in_=x_ap[b, r0:r0 + nr, W - 1:W])
            nc.vector.tensor_copy(out=xt[:, 0:RMAX - 1],
                                  in_=xt[:, RMAX - 1:RMAX].to_broadcast((nr, RMAX - 1)))
            nc.vector.tensor_copy(out=xt[:, RMAX + W + 1:],
                                  in_=xt[:, RMAX + W:RMAX + W + 1].to_broadcast((nr, RMAX - 1)))
            xts.append(xt)

        # horizontal box sums hs[d][j]: [nr, W];  hs_d[:, c] = sum_j xp[:, c+3-d .. c+3+d]
        hs = {1: [], 2: [], 3: []}
        for j, (r0, nr) in enumerate(part_rows):
            xt = xts[j]
            h1 = work_pool.tile([nr, W], F32, tag=f"hs1_{b}_{j}")
            h2 = work_pool.tile([nr, W], F32, tag=f"hs2_{b}_{j}")
            h3 = work_pool.tile([nr, W], F32, tag=f"hs3_{b}_{j}")
            ta = work_pool.tile([nr, W], F32, tag=f"hta_{b}_{j}")
            nc.vector.tensor_tensor(out=ta[:], in0=xt[:, 2:2 + W], in1=xt[:, 3:3 + W], op=Alu.add)
            nc.vector.tensor_tensor(out=h1[:], in0=ta[:], in1=xt[:, 4:4 + W], op=Alu.add)
            nc.vector.tensor_tensor(out=ta[:], in0=xt[:, 1:1 + W], in1=xt[:, 5:5 + W], op=Alu.add)
            nc.vector.tensor_tensor(out=h2[:], in0=h1[:], in1=ta[:], op=Alu.add)
            nc.vector.tensor_tensor(out=ta[:], in0=xt[:, 0:W], in1=xt[:, 6:6 + W], op=Alu.add)
            nc.vector.tensor_tensor(out=h3[:], in0=h2[:], in1=ta[:], op=Alu.add)
            hs[1].append(h1)
            hs[2].append(h2)
            hs[3].append(h3)

        for d in (1, 2, 3):
            area = float((2 * d + 1) ** 2)
            dr0 = (d - 1) * HP  # row base of this (d) block in bmp[b]
            # vertical band matmuls -> psA rows 0..127, psB rows 128..223
            psA = psum_pool.tile([128, H], F32, tag="psA")
            # out rows 0..127 need hs rows 0..130; rows 128..130 sit at
            # partitions 32..34 of the second (rows 96..223) tile.
            nc.tensor.matmul(out=psA[:], lhsT=sdt_lo[:, d - 1, 0:128],
                             rhs=hs[d][0][:], start=True, stop=False)
            nc.tensor.matmul(out=psA[:], lhsT=sdt_hi[32:32 + RMAX, d - 1, 0:128],
                             rhs=hs[d][1][32:32 + RMAX, :], start=False, stop=True)
            psB = psum_pool.tile([96, H], F32, tag="psB")
            # out rows 128..223 need hs rows 125..223 (subset of 96..223);
            # band columns for h>=128 are zero below r=125, so one matmul.
            nc.tensor.matmul(out=psB[:], lhsT=sdt_hi[:, d - 1, 128:224],
                             rhs=hs[d][1][:], start=True, stop=True)

            # scale + column pads -> bmc tiles [nr, HP]
            for j, (ps, nr, r0) in enumerate(((psA, 128, 0), (psB, 96, 128))):
                bmc = work_pool.tile([nr, HP], F32, tag=f"bmc_{b}_{d}_{j}")
                nc.scalar.activation(bmc[:, PAD:PAD + W], ps[:], Act.Copy,
                                     scale=1.0 / area)
                nc.vector.tensor_copy(out=bmc[:, 0:PAD],
                                      in_=bmc[:, PAD:PAD + 1].to_broadcast((nr, PAD)))
                nc.vector.tensor_copy(out=bmc[:, PAD + W:],
                                      in_=bmc[:, PAD + W - 1:PAD + W].to_broadcast((nr, PAD)))
                nc.sync.dma_start(out=bmp[b, dr0 + PAD + r0: dr0 + PAD + r0 + nr, :],
                                  in_=bmc[:])
                if j == 0:
                    ptop = work_pool.tile([PAD, HP], F32, tag=f"ptop_{b}_{d}")
                    nc.gpsimd.partition_broadcast(ptop[:], bmc[0:1, :], PAD)
                    nc.sync.dma_start(out=bmp[b, dr0:dr0 + PAD, :], in_=ptop[:])
                else:
                    brow = work_pool.tile([1, HP], F32, tag=f"brow_{b}_{d}")
                    nc.sync.dma_start(out=brow[:], in_=bmc[nr - 1:nr, :])
                    pbot = work_pool.tile([PAD, HP], F32, tag=f"pbot_{b}_{d}")
                    nc.gpsimd.partition_broadcast(pbot[:], brow[0:1, :], PAD)
                    nc.sync.dma_start(out=bmp[b, dr0 + PAD + H:dr0 + HP, :], in_=pbot[:])

    # ---------------- Stage C: main loop ----------------
    for p in range(P_CORE):
        r1v = nc.values_load(row1[0:1, p:p + 1], engines=[EngT.Activation],
                             min_val=0, max_val=2 * HP + 2 * PAD,
                             skip_runtime_bounds_check=True)
        c1v = nc.values_load(col1[0:1, p:p + 1], engines=[EngT.Activation],
                             min_val=0, max_val=2 * PAD,
                             skip_runtime_bounds_check=True)
        r2v = nc.values_load(row2[0:1, p:p + 1], engines=[EngT.SP],
                             min_val=0, max_val=2 * HP + 2 * PAD,
                             skip_runtime_bounds_check=True)
        c2v = nc.values_load(col2[0:1, p:p + 1], engines=[EngT.SP],
                             min_val=0, max_val=2 * PAD,
                             skip_runtime_bounds_check=True)
        for b in range(B):
            s1 = slab_pool.tile([NPART, NFREE], F32, tag="s1")
            s2 = slab_pool.tile([NPART, NFREE], F32, tag="s2")
            s1v = s1[:].rearrange("k (j w) -> k j w", j=2)
            s2v = s2[:].rearrange("k (j w) -> k j w", j=2)
            nc.scalar.dma_start(out=s1v, in_=bmp[b][ds(r1v, H), ds(c1v, W)])
            nc.sync.dma_start(out=s2v, in_=bmp[b][ds(r2v, H), ds(c2v, W)])
            o = o_pool.tile([NPART, NFREE], F32, tag="o")
            nc.vector.scalar_tensor_tensor(out=o[:], in0=s1[:],
                                           scalar=thr_bc[0:NPART, p:p + 1], in1=s2[:],
                                           op0=Alu.add, op1=Alu.subtract)
            nc.sync.dma_start(out=out_ap[b, p],
                              in_=o[:].rearrange("k (j w) -> k j w", j=2))

    ctx.close()


_COMPILED = {}


def _get_compiled():
    if "nc" not in _COMPILED:
        nc = bacc.Bacc("TRN2", target_bir_lowering=False, debug=False,
                       num_devices=N_CORES)
        build_device_program(nc)
        nc.compile()
        _COMPILED["nc"] = nc
    return _COMPILED["nc"]


def _ensure_ntff_hook():
    """The agent image's antenv lacks axon_hooks; shim it so trace=True can
    drive NTFF profiling via the boot module's ctypes hook (test-only path)."""
    import types

    try:
        from antenv.axon_hooks import get_axon_ntff_profile_hook  # noqa: F401
        return
    except ImportError:
        pass
    import antenv

    mod = types.ModuleType("antenv.axon_hooks")
    _hook = [None]
    mod.set_axon_ntff_profile_hook = lambda h: _hook.__setitem__(0, h)
    mod.get_axon_ntff_profile_hook = lambda: _hook[0]
    sys.modules["antenv.axon_hooks"] = mod
    antenv.axon_hooks = mod
    from trn_agent_boot.trn_boot import _ntff_profile_via_ctypes

    mod.set_axon_ntff_profile_hook(
        _ntff_profile_via_ctypes("/opt/axon/libaxon_pjrt.so"))


def run(inputs: dict, trace: bool = False):
    """Run on the 8 cores. Returns (full output [B,256,H,W], exec_time_ns|None)."""
    x = np.asarray(inputs["x"], dtype=np.float32).reshape(B, H, W)
    offset_x1 = np.asarray(inputs["offset_x1"], np.float32)
    offset_x2 = np.asarray(inputs["offset_x2"], np.float32)
    offset_y1 = np.asarray(inputs["offset_y1"], np.float32)
    offset_y2 = np.asarray(inputs["offset_y2"], np.float32)
    radii = np.asarray(inputs["radii"]).astype(np.int32)
    thresholds = np.asarray(inputs["thresholds"], np.float32)

    sdt = _band_matrices()
    nc = _get_compiled()

    in_maps = []
    for c in range(N_CORES):
        sl = slice(c * P_CORE, (c + 1) * P_CORE)
        in_maps.append({
            "x": x,
            "offy1": offset_y1[sl].reshape(1, P_CORE),
            "offx1": offset_x1[sl].reshape(1, P_CORE),
            "offy2": offset_y2[sl].reshape(1, P_CORE),
            "offx2": offset_x2[sl].reshape(1, P_CORE),
            "radii": radii[sl].reshape(1, P_CORE),
            "thr": thresholds[sl].reshape(1, P_CORE),
            "sdt": sdt,
        })

    if trace:
        _ensure_ntff_hook()
    res = run_bass_kernel_spmd(nc, in_maps, list(range(N_CORES)), trace=trace)
    outs = [res.results[c]["out"] for c in range(N_CORES)]
    return np.concatenate(outs, axis=1), res.exec_time_ns


def kernel(x, offset_x1, offset_x2, offset_y1, offset_y2, radii, thresholds,
           max_radius):
    out, _ = run({
        "x": x, "offset_x1": offset_x1, "offset_x2": offset_x2,
        "offset_y1": offset_y1, "offset_y2": offset_y2,
        "radii": radii, "thresholds": thresholds, "max_radius": max_radius,
    })
    return out


if __name__ == "__main__":
    # smoke test with random data
    rng = np.random.default_rng(0)
    out = kernel(
        x=rng.standard_normal((B, 1, H, W), dtype=np.float32),
        offset_x1=rng.uniform(-16, 16, P_TOTAL).astype(np.float32),
        offset_x2=rng.uniform(-16, 16, P_TOTAL).astype(np.float32),
        offset_y1=rng.uniform(-16, 16, P_TOTAL).astype(np.float32),
        offset_y2=rng.uniform(-16, 16, P_TOTAL).astype(np.float32),
        radii=rng.integers(1, 4, P_TOTAL).astype(np.int32),
        thresholds=(rng.standard_normal(P_TOTAL) * 0.1).astype(np.float32),
        max_radius=3,
    )
    print("out", out.shape, out.dtype, float(np.abs(out).max()))
